# revision 2
# baseline (speedup 1.0000x reference)
"""KernelNorm2d Trainium2 Bass kernel.

Problem: x [16, 64, 256, 256] f32. 2x2 windows (stride 2) over (H, W); per-window
statistics over (C, 2, 2) = 256 elements; out = (x - mean) / sqrt(var + eps),
same shape as x. Data-parallel over batch: 8 cores x 2 samples each.

Per-core layout: partition dim = window-row index i (nH = 128 exactly).
SBUF tile = [128(i), C=64, a=2, WCHUNK=128] where a = row-within-window,
so partition i / free (c, a, w) holds x[b, c, 2*i+a, w0+w].

Per tile (one sample b, one w-chunk):
  - 2 DMA loads (one per a), contiguous 512B runs in DRAM.
  - Sum over window (c, a, b2) keeping j = w//2: one tensor_reduce over the
    4D view [p, j(64), ca(128), b2(2)], axis=XY (DVE).
  - Sum of squares: ACT squares (a, w-half) chunks into a small scratch,
    DVE tensor_reduce per chunk, add the two a-halves (DVE).
  - Small [p,64] ops to get inv_std and shift t = -mean*inv_std.
  - Normalize per window column j: x*inv + t, split across ACT
    (activation Identity w/ per-partition scale+bias) and GPSIMD
    (tensor_scalar), in place.
  - 2 DMA stores.
"""

import os
import sys

for _p in ("/opt/trn_rl_repo", "/root/.axon_site/_ro/trn_rl_repo"):
    if os.path.isdir(_p) and _p not in sys.path:
        sys.path.append(_p)

import numpy as np

import concourse.bass as bass
import concourse.tile as tile
from concourse import bacc, mybir
from concourse.bass_utils import run_bass_kernel_spmd

# Problem constants (hardcoded per spec nn_KernelNorm2d_72164040507639)
B, C, H, W = 16, 64, 256, 256
N_CORES = 8
B_LOC = B // N_CORES          # samples per core
NH = H // 2                   # 128 window rows = partition dim
EPS = 1e-5
WCHUNK = 64                   # w elements per tile
NJ = WCHUNK // 2              # 64 windows per tile per partition
NWC = W // WCHUNK             # 2 w-chunks
WIN = C * 4                   # 256 elements per window


def _norm_engine(nc, j):
    """Normalize-instruction engine for window column j (load balancing)."""
    return nc.scalar if j % 2 == 0 else nc.gpsimd


def build_kernel(
    b_loc: int = B_LOC,
    c_dim: int = C,
    h_dim: int = H,
    w_dim: int = W,
    wchunk: int = WCHUNK,
    debug: bool = False,
) -> bass.Bass:
    B_LOC, C, H, W, WCHUNK = b_loc, c_dim, h_dim, w_dim, wchunk
    NH = H // 2
    NJ = WCHUNK // 2
    NJH = NJ // 2                 # windows per w-half chunk
    WH = WCHUNK // 2              # w elems per half chunk
    NWC = W // WCHUNK
    WIN = C * 4
    nc = bacc.Bacc("TRN2", debug=debug)
    x = nc.dram_tensor("x", [B_LOC, C, H, W], mybir.dt.float32, kind="ExternalInput")
    y = nc.dram_tensor("y", [B_LOC, C, H, W], mybir.dt.float32, kind="ExternalOutput")

    with tile.TileContext(nc) as tc:
        with (
            tc.tile_pool(name="data", bufs=4) as data_pool,
            tc.tile_pool(name="stats", bufs=4) as stats_pool,
            tc.tile_pool(name="scratch", bufs=2) as scratch_pool,
            tc.tile_pool(name="singles", bufs=1) as singles,
        ):
            eps_tile = singles.tile([NH, 1], mybir.dt.float32)
            nc.vector.memset(eps_tile, EPS)
            for b in range(B_LOC):
                for wc in range(NWC):
                    ws = wc * WCHUNK
                    xt = data_pool.tile([NH, C, 2, WCHUNK], mybir.dt.float32)

                    # ---- load: per a, [i, c, w] <- x[b, :, a::2, ws:ws+WCHUNK]
                    for a in range(2):
                        asrc = x[b, :, a::2, ws : ws + WCHUNK].transpose([1, 0, 2])
                        nc.sync.dma_start(out=xt[:, :, a, :], in_=asrc)

                    # 4D window view [p, j, ca, b2]
                    xt4 = xt.rearrange("p c a (j b2) -> p j (c a) b2", b2=2)

                    # ---- window sums (DVE, one pass)
                    s_sum = stats_pool.tile([NH, NJ], mybir.dt.float32, tag="s_sum")
                    nc.vector.tensor_reduce(
                        out=s_sum,
                        in_=xt4,
                        axis=mybir.AxisListType.XY,
                        op=mybir.AluOpType.add,
                    )

                    # ---- window sums of squares: ACT square chunks + DVE reduce
                    qa0 = stats_pool.tile([NH, NJ], mybir.dt.float32, tag="qa0")
                    qa1 = stats_pool.tile([NH, NJ], mybir.dt.float32, tag="qa1")
                    qa = [qa0, qa1]
                    for a in range(2):
                        for wh in range(2):
                            x2 = scratch_pool.tile([NH, C, WH], mybir.dt.float32, tag="x2")
                            nc.scalar.activation(
                                out=x2,
                                in_=xt[:, :, a, wh * WH : (wh + 1) * WH],
                                func=mybir.ActivationFunctionType.Square,
                            )
                            x2v = x2.rearrange("p c (j b2) -> p j c b2", b2=2)
                            nc.vector.tensor_reduce(
                                out=qa[a][:, wh * NJH : (wh + 1) * NJH],
                                in_=x2v,
                                axis=mybir.AxisListType.XY,
                                op=mybir.AluOpType.add,
                            )
                    q_sum = stats_pool.tile([NH, NJ], mybir.dt.float32, tag="q_sum")
                    nc.vector.tensor_add(out=q_sum, in0=qa[0], in1=qa[1])

                    # ---- stats: inv = 1/sqrt(E[x^2] - mean^2 + eps), t = -mean*inv
                    nm = stats_pool.tile([NH, NJ], mybir.dt.float32, tag="nm")
                    var = stats_pool.tile([NH, NJ], mybir.dt.float32, tag="var")
                    nm2 = stats_pool.tile([NH, NJ], mybir.dt.float32, tag="nm2")
                    inv = stats_pool.tile([NH, NJ], mybir.dt.float32, tag="inv")
                    tsh = stats_pool.tile([NH, NJ], mybir.dt.float32, tag="tsh")

                    nc.vector.tensor_scalar_mul(out=nm, in0=s_sum, scalar1=-1.0 / WIN)
                    nc.vector.tensor_mul(out=nm2, in0=nm, in1=nm)
                    nc.vector.tensor_scalar_mul(out=var, in0=q_sum, scalar1=1.0 / WIN)
                    nc.vector.tensor_tensor(
                        out=var, in0=var, in1=nm2, op=mybir.AluOpType.subtract
                    )
                    nc.scalar.activation(
                        out=var,
                        in_=var,
                        func=mybir.ActivationFunctionType.Sqrt,
                        bias=eps_tile,
                        scale=1.0,
                    )
                    nc.vector.reciprocal(out=inv, in_=var)
                    nc.vector.tensor_mul(out=tsh, in0=nm, in1=inv)

                    # ---- normalize in place: x*inv + t, ACT/GPSIMD split
                    for j in range(NJ):
                        win = xt4[:, j, :, :]
                        if j % 2 == 0:
                            nc.scalar.activation(
                                out=win,
                                in_=win,
                                func=mybir.ActivationFunctionType.Identity,
                                bias=tsh[:, j : j + 1],
                                scale=inv[:, j : j + 1],
                            )
                        else:
                            nc.gpsimd.tensor_scalar(
                                out=win,
                                in0=win,
                                scalar1=inv[:, j : j + 1],
                                scalar2=tsh[:, j : j + 1],
                                op0=mybir.AluOpType.mult,
                                op1=mybir.AluOpType.add,
                            )

                    # ---- store
                    for a in range(2):
                        adst = y[b, :, a::2, ws : ws + WCHUNK].transpose([1, 0, 2])
                        nc.sync.dma_start(out=adst, in_=xt[:, :, a, :])
    nc.compile()
    return nc


_NC_CACHE = None
LAST_RESULTS = None


def _get_nc():
    global _NC_CACHE
    if _NC_CACHE is None:
        _NC_CACHE = build_kernel()
    return _NC_CACHE


def kernel(x: np.ndarray) -> np.ndarray:
    global LAST_RESULTS
    assert x.shape == (B, C, H, W), x.shape
    x = np.ascontiguousarray(x, dtype=np.float32)
    nc = _get_nc()
    in_maps = [
        {"x": x[k * B_LOC : (k + 1) * B_LOC]} for k in range(N_CORES)
    ]
    kw = {}
    if os.environ.get("KERNEL_TRACE") == "1":
        kw["trace"] = True
        if os.environ.get("KERNEL_TRACE_DIR"):
            kw["tmpdir"] = os.environ["KERNEL_TRACE_DIR"]
    res = run_bass_kernel_spmd(nc, in_maps, core_ids=list(range(N_CORES)), **kw)
    LAST_RESULTS = res
    out = np.concatenate([r["y"] for r in res.results], axis=0)
    return out

